# revision 4
# baseline (speedup 1.0000x reference)
"""Distributed Trainium2 (Bass) kernel for the gnn_message_passing problem.

Strategy (data-parallel over the B=64 equal graphs):
  - core c owns nodes [c*2048, (c+1)*2048) (8 graphs); edges are partitioned
    by the core that owns their *destination* and sorted by dst, so the
    segment softmax / scatter-add becomes a block-local one-hot matmul
    (fp8 one-hot adjacency blocks on the TensorEngine, fp32 PSUM accum).
  - per layer, the xk/xv node tables (bf16) are AllGather'd across the 8
    cores; per-edge k/v rows are fetched from the gathered table with SWDGE
    dma_gather (1KB rows); q rows are gathered from a core-local table.
  - edge features (epa @ We^T) are computed on the fly per 128-edge chunk
    with the TensorEngine; softmax runs without max-subtraction (activations
    are O(1) here; exp is evaluated in fp32 on ScalarE).

Host-side work (sharding, sorting, padding, bf16 casts, output assembly) is
numpy; all FLOP-heavy work runs on the 8 NeuronCores.
"""
import dataclasses
import os
import sys
import types
import contextlib
import ctypes
from contextlib import ExitStack

import numpy as np
import ml_dtypes

# hardcoded problem geometry (spec: nn_AC_88399016886986)
N, E, B, MAXN = 16384, 262144, 64, 256
D, HC, DFF, L = 256, 256, 1024, 3
H, HD = 8, 32
NCORES = 8

_CACHE = {}


# ==========================================================================
# harness patches (this container's walrus accepts only one inline sync wait
# per instruction; NTFF profiling hook is missing from the image's antenv)
# ==========================================================================
def _apply_tile_patch():
    import concourse.tile as tile
    from concourse import mybir
    from concourse.vector_clock import ScopedClock

    def _drain_and_barrier(self, tick_clock, wait_clock):
        probe = self.nc.sync.nop(nofuse=True)
        wait_clock.add_sem_waits(probe.ins, ScopedClock({None: tick_clock.global_clock}))
        si = probe.ins.sync_info
        waits = list(si.on_wait) if si and si.on_wait else []
        probe.ins.sync_info = mybir.SyncInfo(on_wait=waits[:1], on_update=[])
        for w in waits[1:]:
            n = self.nc.sync.nop(nofuse=True)
            n.ins.sync_info = mybir.SyncInfo(on_wait=[w], on_update=[])
        self.nc.sync.drain()
        self.nc.all_engine_barrier()
        assert self.sems is not None
        popped = self.nc._tile_sem_poison_stack.pop()
        assert popped is self._sem_poison
        self.nc.clear_and_free_semaphores(list(self.sems.allocated().values()))
        self.nc.all_engine_barrier()

    tile.TileContext._drain_and_barrier = _drain_and_barrier


def _apply_ntff_shim():
    if "antenv.axon_hooks" in sys.modules:
        return
    _SO = "/opt/axon/libaxon_pjrt.so"
    holder = [None]

    def _make_hook():
        try:
            lib = ctypes.CDLL(_SO)
        except OSError:
            return None
        if not hasattr(lib, "axon_start_nrt_profile"):
            return None
        lib.axon_start_nrt_profile.argtypes = [ctypes.POINTER(ctypes.c_int64), ctypes.c_size_t]
        lib.axon_start_nrt_profile.restype = ctypes.c_int64
        lib.axon_stop_nrt_profile.argtypes = [ctypes.c_char_p]
        lib.axon_stop_nrt_profile.restype = ctypes.c_int64

        @contextlib.contextmanager
        def _hook(output_dir, device_ids):
            import jax
            jax.devices()
            if device_ids:
                ids = (ctypes.c_int64 * len(device_ids))(*device_ids)
                rc = lib.axon_start_nrt_profile(ids, len(device_ids))
            else:
                rc = lib.axon_start_nrt_profile(None, 0)
            if rc != 0:
                raise RuntimeError(f"axon_start_nrt_profile rc={rc}")
            try:
                yield
            finally:
                nf = lib.axon_stop_nrt_profile(str(output_dir).encode())
                print(f"ntff profile: {nf} file(s) written to {output_dir}", file=sys.stderr)
        return _hook

    mod = types.ModuleType("antenv.axon_hooks")
    mod.get_axon_ntff_profile_hook = lambda: holder[0]
    mod.set_axon_ntff_profile_hook = lambda h: holder.__setitem__(0, h)
    sys.modules["antenv.axon_hooks"] = mod
    holder[0] = _make_hook()
    import concourse.bass_utils as bu
    bu.upload_artifacts = lambda tmpdir: "local://" + str(tmpdir)


def _split_sync_waits(nc, mybir, maxw=1):
    for f in nc.m.functions:
        for bb in f.blocks:
            newlist = []
            for inst in bb.instructions:
                si = inst.sync_info
                waits = list(si.on_wait) if si and si.on_wait else []
                if len(waits) > maxw:
                    extra, keep = waits[:-maxw], waits[-maxw:]
                    for i, w in enumerate(extra):
                        nop = mybir.InstNoOp(name=f"{inst.name}-ws{i}", ins=[], outs=[])
                        nop.engine = inst.engine
                        nop.sync_info = mybir.SyncInfo(on_wait=[w], on_update=[])
                        newlist.append(nop)
                        nc.register_instruction(nop, overwrite=True)
                    inst.sync_info = mybir.SyncInfo(
                        on_wait=keep,
                        on_update=list(si.on_update) if si.on_update else [])
                newlist.append(inst)
            bb.instructions[:] = newlist


# ==========================================================================
# host prep
# ==========================================================================
def _bf(x):
    return np.ascontiguousarray(np.asarray(x, np.float32)).astype(ml_dtypes.bfloat16)


def _wrap_idx(idx_flat):
    M = idx_flat.shape[0]
    w = idx_flat.reshape(M // 16, 16).T.astype(np.int16)
    return np.tile(w, (8, 1))


def _prep(inp):
    SCALE = 1.0 / np.sqrt(HD)
    NL = N // NCORES
    NT = NL // 128
    src = np.asarray(inp['edge_index'][0])
    dst = np.asarray(inp['edge_index'][1])
    epa = np.asarray(inp['edge_power_attn'], np.float32).reshape(E, -1)
    EDIM = epa.shape[1]

    res = np.concatenate([np.asarray(inp['power_alloc'], np.float32),
                          np.asarray(inp['beam_alloc'], np.float32)], axis=2).reshape(N, -1)
    npa = np.asarray(inp['node_power_attn'], np.float32).reshape(N, -1)

    cores = []
    ET = 0
    for c in range(NCORES):
        lo, hi = c * NL, (c + 1) * NL
        eids = np.nonzero((dst >= lo) & (dst < hi))[0]
        dl = dst[eids] - lo
        order = np.argsort(dl, kind='stable')
        eids, dl = eids[order], dl[order]
        counts = np.bincount(dl // 128, minlength=NT)
        ET = max(ET, int(counts.max()))
        cores.append((eids, dl, counts))
    ET = int(np.ceil(ET / 512) * 512)
    CT = ET // 128

    percore = []
    for c in range(NCORES):
        eids, dl, counts = cores[c]
        src_g = np.zeros((NT, ET), np.int64)
        qd_l = np.zeros((NT, ET), np.int64)
        A = np.zeros((NT, CT, 128, 128), np.float32)
        epaT = np.zeros((NT, EDIM, ET), np.float32)
        pos = 0
        for t in range(NT):
            cnt = counts[t]
            e_t = eids[pos:pos + cnt]
            d_t = dl[pos:pos + cnt]
            pos += cnt
            src_g[t, :cnt] = src[e_t]
            qd_l[t, :cnt] = d_t
            sl = np.arange(cnt)
            A[t, sl // 128, sl % 128, d_t - t * 128] = 1.0
            epaT[t, :, :cnt] = epa[e_t].T
        percore.append({
            'A': A.astype(ml_dtypes.float8_e4m3fn),
            'epaT': _bf(epaT),
            'srcidx': np.concatenate([_wrap_idx(src_g[t]) for t in range(NT)], axis=1),
            'qdstidx': np.concatenate([_wrap_idx(qd_l[t]) for t in range(NT)], axis=1),
            'resT': _bf(res[c * NL:(c + 1) * NL].T),
            'npaT': _bf(npa[c * NL:(c + 1) * NL].T),
        })

    w = {}
    w['WinTa'] = _bf(np.asarray(inp['W_in']).T[:128])
    w['WinTb'] = _bf(np.asarray(inp['W_in']).T[128:])
    w['bin'] = _bf(np.asarray(inp['b_in'])[None, :])
    w['WembTa'] = _bf(np.asarray(inp['W_emb']).T[:128])
    w['WembTb'] = _bf(np.asarray(inp['W_emb']).T[128:])
    w['bemb'] = _bf(np.asarray(inp['b_emb'])[None, :])
    w['WqT'] = _bf(np.asarray(inp['Wq']).transpose(0, 2, 1) * SCALE)
    w['bq'] = _bf(np.asarray(inp['bq'])[:, None, :] * SCALE)
    w['WkT'] = _bf(np.asarray(inp['Wk']).transpose(0, 2, 1))
    w['bk'] = _bf(np.asarray(inp['bk'])[:, None, :])
    w['WvT'] = _bf(np.asarray(inp['Wv']).transpose(0, 2, 1))
    w['bv'] = _bf(np.asarray(inp['bv'])[:, None, :])
    w['WskT'] = _bf(np.asarray(inp['Wskip']).transpose(0, 2, 1))
    w['bsk'] = _bf(np.asarray(inp['bskip'])[:, None, :])
    WeT = np.asarray(inp['We']).transpose(0, 2, 1)
    w['WeTa'] = _bf(WeT[:, :128])
    w['WeTb'] = _bf(WeT[:, 128:])
    w['Wf1T'] = _bf(np.asarray(inp['Wf1']).transpose(0, 2, 1))
    bf1 = np.asarray(inp['bf1'])
    w['bf1p'] = np.ascontiguousarray(
        bf1.reshape(L, DFF // 128, 128).transpose(0, 2, 1)).astype(np.float32)
    w['Wf2T'] = _bf(np.asarray(inp['Wf2']).transpose(0, 2, 1))
    w['bf2'] = _bf(np.asarray(inp['bf2'])[:, None, :])
    for nm in ('ln1_g', 'ln1_b', 'ln2_g', 'ln2_b'):
        w[nm] = _bf(np.tile(np.asarray(inp[nm])[:, None, :], (1, 128, 1)))
    w['Wlinkr'] = _bf(np.tile(np.asarray(inp['W_link'])[0][None, :], (128, 1)))
    w['Wcritr'] = _bf(np.tile(np.asarray(inp['W_critic'])[0][None, :], (128, 1)))
    w['ident'] = _bf(np.eye(128, dtype=np.float32))
    w['ones'] = _bf(np.ones((1, 128), np.float32))

    cfg = dict(NL=NL, NT=NT, ET=ET, CT=CT, EDIM=EDIM, INDIM=res.shape[1])
    return cfg, w, percore


# ==========================================================================
# bass graph
# ==========================================================================
def _build(cfg):
    import concourse.bass as bass
    import concourse.tile as tile
    from concourse import mybir
    from concourse import library_config

    BF16 = mybir.dt.bfloat16
    F32 = mybir.dt.float32
    FP8 = mybir.dt.float8e4
    I16 = mybir.dt.int16

    def bcast_last(ap, n):
        return dataclasses.replace(ap, ap=list(ap.ap) + [[0, n]])

    NL, NT, ET, CT = cfg['NL'], cfg['NT'], cfg['ET'], cfg['CT']
    EDIM = cfg['EDIM']
    EB = EDIM - 128
    GRP = 4
    assert CT % GRP == 0
    NG = CT // GRP
    IDXW = ET // 16
    NTG = 2

    nc = bass.Bass("TRN2", target_bir_lowering=False, debug=False, num_devices=NCORES)

    def par(name, shape, dt):
        return nc.declare_dram_parameter(name, list(shape), dt, isOutput=False)

    A_d = par('A', (NT, CT, 128, 128), FP8)
    epaT_d = par('epaT', (NT, EDIM, ET), BF16)
    srcidx_d = par('srcidx', (128, NT * IDXW), I16)
    qdstidx_d = par('qdstidx', (128, NT * IDXW), I16)
    resT_d = par('resT', (cfg['INDIM'], NL), BF16)
    npaT_d = par('npaT', (cfg['INDIM'], NL), BF16)
    wd = {}
    for nm, shape, dt in [
        ('WinTa', (128, D), BF16), ('WinTb', (cfg['INDIM'] - 128, D), BF16), ('bin', (1, D), BF16),
        ('WembTa', (128, D), BF16), ('WembTb', (cfg['INDIM'] - 128, D), BF16), ('bemb', (1, D), BF16),
        ('WqT', (L, D, HC), BF16), ('bq', (L, 1, HC), BF16),
        ('WkT', (L, D, HC), BF16), ('bk', (L, 1, HC), BF16),
        ('WvT', (L, D, HC), BF16), ('bv', (L, 1, HC), BF16),
        ('WskT', (L, D, HC), BF16), ('bsk', (L, 1, HC), BF16),
        ('WeTa', (L, 128, HC), BF16), ('WeTb', (L, EB, HC), BF16),
        ('Wf1T', (L, D, DFF), BF16), ('bf1p', (L, 128, DFF // 128), F32),
        ('Wf2T', (L, DFF, D), BF16), ('bf2', (L, 1, D), BF16),
        ('ln1_g', (L, 128, D), BF16), ('ln1_b', (L, 128, D), BF16),
        ('ln2_g', (L, 128, D), BF16), ('ln2_b', (L, 128, D), BF16),
        ('Wlinkr', (128, D), BF16), ('Wcritr', (128, D), BF16),
        ('ident', (128, 128), BF16), ('ones', (1, 128), BF16),
    ]:
        wd[nm] = par(nm, shape, dt)

    h_out = nc.declare_dram_parameter('h_out', [NL, D], F32, isOutput=True)
    link_out = nc.declare_dram_parameter('link_out', [NL], F32, isOutput=True)
    crit_out = nc.declare_dram_parameter('crit_out', [NL], F32, isOutput=True)

    xq_dram = nc.dram_tensor('xq_dram', [NL, HC], BF16)
    cc_in = nc.dram_tensor('cc_in', [NL, 2 * HC], BF16)
    xkv_full = nc.dram_tensor('xkv_full', [N, 2 * HC], BF16, addr_space="Shared")

    n2p = lambda ap: ap.rearrange("(t p) f -> p t f", p=128)

    with ExitStack() as ctx:
        tc = ctx.enter_context(tile.TileContext(nc))
        nc.gpsimd.load_library(library_config.mlp)

        const = ctx.enter_context(tc.tile_pool(name="const", bufs=1))
        wpool = ctx.enter_context(tc.tile_pool(name="wpool", bufs=1))
        state = ctx.enter_context(tc.tile_pool(name="state", bufs=1))
        nwork = ctx.enter_context(tc.tile_pool(name="nwork", bufs=2))
        small = ctx.enter_context(tc.tile_pool(name="small", bufs=1))
        ps_n = ctx.enter_context(tc.tile_pool(name="ps_n", bufs=3, space="PSUM"))
        ps_ee = ctx.enter_context(tc.tile_pool(name="ps_ee", bufs=2, space="PSUM"))
        ps_ag = ctx.enter_context(tc.tile_pool(name="ps_ag", bufs=1, space="PSUM"))

        dma = nc.sync.dma_start

        ident = const.tile([128, 128], BF16)
        dma(ident[:], wd['ident'][:])
        ones = const.tile([1, 128], BF16)
        dma(ones[:], wd['ones'][:])
        srcidx = const.tile([128, NT * IDXW], I16)
        dma(srcidx[:], srcidx_d[:])
        qdstidx = const.tile([128, NT * IDXW], I16)
        dma(qdstidx[:], qdstidx_d[:])
        Wlk = const.tile([128, D], BF16)
        dma(Wlk[:], wd['Wlinkr'][:])
        Wcr = const.tile([128, D], BF16)
        dma(Wcr[:], wd['Wcritr'][:])

        x_sb = state.tile([128, NT, D], BF16)
        inp0_sb = state.tile([128, NT, D], BF16)
        xin_sb = state.tile([128, NT, D], BF16)
        xinT_sb = state.tile([128, 2, NT, 128], BF16)
        xsk_sb = state.tile([128, NT, HC], BF16)
        xln1_sb = state.tile([128, NT, D], BF16)
        xln1T_sb = state.tile([128, 2, NT, 128], BF16)
        link_sb = state.tile([128, NT], F32)
        crit_sb = state.tile([128, NT], F32)

        def layernorm_to(dst_ap, u, gam, bet, extra_f32=None):
            mu = small.tile([128, 1], F32, tag="mu")
            nc.vector.reduce_sum(mu[:], u[:], axis=mybir.AxisListType.X)
            nc.vector.tensor_scalar_mul(mu[:], mu[:], 1.0 / D)
            cen = small.tile([128, D], F32, tag="cen")
            nc.vector.tensor_scalar(cen[:], u[:], mu[:], None, mybir.AluOpType.subtract)
            sq = small.tile([128, D], F32, tag="sq")
            nc.scalar.square(sq[:], cen[:])
            var = small.tile([128, 1], F32, tag="var")
            nc.vector.reduce_sum(var[:], sq[:], axis=mybir.AxisListType.X)
            nc.vector.tensor_scalar(var[:], var[:], 1.0 / D, 1e-5,
                                    mybir.AluOpType.mult, mybir.AluOpType.add)
            rstd = small.tile([128, 1], F32, tag="rstd")
            nc.scalar.sqrt(rstd[:], var[:])
            nc.vector.reciprocal(rstd[:], rstd[:])
            xl = small.tile([128, D], BF16, tag="xl")
            nc.vector.tensor_scalar_mul(xl[:], cen[:], rstd[:])
            nc.vector.tensor_mul(xl[:], xl[:], gam[:])
            nc.vector.tensor_add(dst_ap, xl[:], bet[:])
            if extra_f32 is not None:
                xf = small.tile([128, D], F32, tag="xf")
                nc.vector.tensor_add(xf[:], xl[:], bet[:])
                dma(extra_f32, xf[:])

        # phase 0: input embeddings (pool freed before the edge-phase pools)
        embpool = tc.tile_pool(name="emb", bufs=1)
        emb = embpool.__enter__()
        emb_in = emb.tile([128, 2, NL], BF16)
        embB_in = emb.tile([EB, 2, NL], BF16)
        dma(emb_in[:, 0, :], resT_d[0:128, :])
        dma(embB_in[:, 0, :], resT_d[128:EDIM, :])
        dma(emb_in[:, 1, :], npaT_d[0:128, :])
        dma(embB_in[:, 1, :], npaT_d[128:EDIM, :])
        embW = emb.tile([128, 2, D], BF16)
        embWB = emb.tile([EB, 2, D], BF16)
        dma(embW[:, 0, :], wd['WinTa'][:])
        dma(embWB[:, 0, :], wd['WinTb'][:])
        dma(embW[:, 1, :], wd['WembTa'][:])
        dma(embWB[:, 1, :], wd['WembTb'][:])
        embb = emb.tile([1, 2, D], BF16)
        dma(embb[:, 0, :], wd['bin'][:])
        dma(embb[:, 1, :], wd['bemb'][:])
        for t in range(NT):
            for j, dstt in ((0, inp0_sb), (1, x_sb)):
                ps = ps_n.tile([128, 512], F32, tag="psn")
                nc.tensor.matmul(ps[:, 0:D], emb_in[:, j, t * 128:(t + 1) * 128], embW[:, j, :], start=True, stop=False)
                nc.tensor.matmul(ps[:, 0:D], embB_in[:, j, t * 128:(t + 1) * 128], embWB[:, j, :], start=False, stop=False)
                nc.tensor.matmul(ps[:, 0:D], ones[:], embb[:, j, :], start=False, stop=True)
                nc.scalar.copy(dstt[:, t, :], ps[:, 0:D])
        embpool.__exit__(None, None, None)
        ework = ctx.enter_context(tc.tile_pool(name="ework", bufs=2))
        gwork = ctx.enter_context(tc.tile_pool(name="gwork", bufs=2))

        for l in range(L):
            WqT = wpool.tile([128, 2, HC], BF16, tag="wq")
            WkT = wpool.tile([128, 2, HC], BF16, tag="wk")
            WvT = wpool.tile([128, 2, HC], BF16, tag="wv")
            WskT = wpool.tile([128, 2, HC], BF16, tag="wsk")
            for tl, nm in ((WqT, 'WqT'), (WkT, 'WkT'), (WvT, 'WvT'), (WskT, 'WskT')):
                dma(tl[:], wd[nm][l].rearrange("(c p) f -> p c f", p=128))
            biases = wpool.tile([1, 4, HC], BF16, tag="wb")
            for j, nm in enumerate(('bq', 'bk', 'bv', 'bsk')):
                dma(biases[:, j, :], wd[nm][l])
            WeA = wpool.tile([128, HC], BF16, tag="wea")
            dma(WeA[:], wd['WeTa'][l])
            WeB = wpool.tile([EB, HC], BF16, tag="web")
            dma(WeB[:], wd['WeTb'][l])
            Wf1 = wpool.tile([128, 2, DFF], BF16, tag="wf1")
            dma(Wf1[:], wd['Wf1T'][l].rearrange("(c p) f -> p c f", p=128))
            bf1p = wpool.tile([128, DFF // 128], F32, tag="wbf1")
            dma(bf1p[:], wd['bf1p'][l])
            Wf2 = wpool.tile([128, DFF // 128, D], BF16, tag="wf2")
            dma(Wf2[:], wd['Wf2T'][l].rearrange("(c p) f -> p c f", p=128))
            bf2 = wpool.tile([1, D], BF16, tag="wbf2")
            dma(bf2[:], wd['bf2'][l])
            g1 = wpool.tile([128, D], BF16, tag="g1")
            dma(g1[:], wd['ln1_g'][l])
            b1 = wpool.tile([128, D], BF16, tag="b1")
            dma(b1[:], wd['ln1_b'][l])
            g2 = wpool.tile([128, D], BF16, tag="g2")
            dma(g2[:], wd['ln2_g'][l])
            b2 = wpool.tile([128, D], BF16, tag="b2")
            dma(b2[:], wd['ln2_b'][l])

            # node phase
            nc.vector.tensor_add(xin_sb[:], x_sb[:], inp0_sb[:])
            for t in range(NT):
                for dc in range(2):
                    pst = ps_n.tile([128, 512], BF16, tag="psn")
                    nc.tensor.transpose(pst[:, 0:128], xin_sb[:, t, dc * 128:(dc + 1) * 128], ident[:])
                    nc.scalar.copy(xinT_sb[:, dc, t, :], pst[:, 0:128])
            for t in range(NT):
                kvt = nwork.tile([128, 2 * HC], BF16, tag="kvt")
                xqt = nwork.tile([128, HC], BF16, tag="xqt")
                for j, W in ((0, WqT), (3, WskT), (1, WkT), (2, WvT)):
                    ps = ps_n.tile([128, 512], F32, tag="psn")
                    nc.tensor.matmul(ps[:, 0:HC], xinT_sb[:, 0, t, :], W[:, 0, :], start=True, stop=False)
                    nc.tensor.matmul(ps[:, 0:HC], xinT_sb[:, 1, t, :], W[:, 1, :], start=False, stop=False)
                    nc.tensor.matmul(ps[:, 0:HC], ones[:], biases[:, j, :], start=False, stop=True)
                    if j == 0:
                        nc.scalar.copy(xqt[:], ps[:, 0:HC])
                    elif j == 3:
                        nc.scalar.copy(xsk_sb[:, t, :], ps[:, 0:HC])
                    elif j == 1:
                        nc.vector.tensor_copy(kvt[:, 0:HC], ps[:, 0:HC])
                    else:
                        nc.vector.tensor_copy(kvt[:, HC:2 * HC], ps[:, 0:HC])
                dma(n2p(cc_in.ap())[:, t, :], kvt[:])
                dma(n2p(xq_dram.ap())[:, t, :], xqt[:])
            nc.gpsimd.collective_compute(
                "AllGather", mybir.AluOpType.bypass,
                replica_groups=[list(range(NCORES))],
                ins=[cc_in.ap().opt()], outs=[xkv_full.ap().opt()])

            # edge phase
            et_reg = nc.gpsimd.snap(ET)
            for t in range(NT):
                kvg = gwork.tile([128, CT, 2 * HC], BF16, tag="kvg")
                nc.gpsimd.dma_gather(
                    out_ap=kvg[:], in_ap=xkv_full.ap(),
                    idxs_ap=srcidx[:, t * IDXW:(t + 1) * IDXW],
                    num_idxs=ET, num_idxs_reg=et_reg, elem_size=2 * HC, single_packet=False)
                qg = gwork.tile([128, CT, HC], BF16, tag="qg")
                nc.gpsimd.dma_gather(
                    out_ap=qg[:], in_ap=xq_dram.ap(),
                    idxs_ap=qdstidx[:, t * IDXW:(t + 1) * IDXW],
                    num_idxs=ET, num_idxs_reg=et_reg, elem_size=HC, single_packet=False)
                epaA = gwork.tile([128, ET], BF16, tag="epaA")
                dma(epaA[:], epaT_d[t, 0:128, :])
                epaB = gwork.tile([EB, ET], BF16, tag="epaB")
                dma(epaB[:], epaT_d[t, 128:EDIM, :])
                A_t = gwork.tile([128, CT, 128], FP8, tag="At")
                dma(A_t[:], A_d[t].rearrange("c p n -> p c n"))

                agg = ps_ag.tile([128, HC + H], F32, tag="agg")
                for g in range(NG):
                    ee = ps_ee.tile([128, GRP, HC], F32, tag="ee")
                    for j in range(GRP):
                        cch = g * GRP + j
                        sl = slice(cch * 128, (cch + 1) * 128)
                        nc.tensor.matmul(ee[:, j, :], epaA[:, sl], WeA[:], start=True, stop=False)
                        nc.tensor.matmul(ee[:, j, :], epaB[:, sl], WeB[:], start=False, stop=True)
                    gs = slice(g * GRP, (g + 1) * GRP)
                    k_sb = ework.tile([128, GRP, HC], BF16, tag="k")
                    nc.vector.tensor_add(k_sb[:], ee[:], kvg[:, gs, 0:HC])
                    v_sb = ework.tile([128, GRP, HC], BF16, tag="v")
                    nc.vector.tensor_add(v_sb[:], ee[:], kvg[:, gs, HC:2 * HC])
                    prod = k_sb
                    nc.vector.tensor_mul(prod[:], k_sb[:], qg[:, gs, :])
                    alph = ework.tile([128, GRP, H], F32, tag="alph")
                    nc.vector.reduce_sum(
                        alph[:],
                        prod[:].rearrange("p g (h d) -> p g h d", h=H),
                        axis=mybir.AxisListType.X)
                    wex = ework.tile([128, GRP, HC + H], BF16, tag="wex")
                    nc.scalar.activation(wex[:, :, HC:HC + H], alph[:],
                                         mybir.ActivationFunctionType.Exp)
                    nc.vector.tensor_mul(
                        wex[:, :, 0:HC].rearrange("p g (h d) -> p g h d", h=H),
                        v_sb[:].rearrange("p g (h d) -> p g h d", h=H),
                        bcast_last(wex[:, :, HC:HC + H], HD))
                    for j in range(GRP):
                        cch = g * GRP + j
                        nc.tensor.matmul(agg[:], A_t[:, cch, :], wex[:, j, :],
                                         start=(cch == 0), stop=(cch == CT - 1))

                dinv = small.tile([128, H], F32, tag="dinv")
                nc.vector.tensor_scalar_add(dinv[:], agg[:, HC:HC + H], 1e-16)
                nc.vector.reciprocal(dinv[:], dinv[:])
                u = small.tile([128, D], F32, tag="u")
                nc.vector.tensor_mul(
                    u[:].rearrange("p (h d) -> p h d", h=H),
                    agg[:, 0:HC].rearrange("p (h d) -> p h d", h=H),
                    bcast_last(dinv[:], HD))
                nc.vector.tensor_add(u[:], u[:], xsk_sb[:, t, :])
                nc.vector.tensor_add(u[:], u[:], xin_sb[:, t, :])
                layernorm_to(xln1_sb[:, t, :], u, g1, b1)
                for dc in range(2):
                    pst = ps_n.tile([128, 512], BF16, tag="psn")
                    nc.tensor.transpose(pst[:, 0:128], xln1_sb[:, t, dc * 128:(dc + 1) * 128], ident[:])
                    nc.scalar.copy(xln1T_sb[:, dc, t, :], pst[:, 0:128])

            # FFN
            for ng in range(NT // NTG):
                h1T = nwork.tile([128, DFF // 128, NTG * 128], BF16, tag="h1T")
                for fc in range(DFF // 128):
                    h1 = ps_n.tile([128, 512], F32, tag="psn")
                    nc.tensor.matmul(h1[:, 0:NTG * 128], Wf1[:, 0, fc * 128:(fc + 1) * 128],
                                     xln1T_sb[:, 0, ng * NTG:(ng + 1) * NTG, :], start=True, stop=False)
                    nc.tensor.matmul(h1[:, 0:NTG * 128], Wf1[:, 1, fc * 128:(fc + 1) * 128],
                                     xln1T_sb[:, 1, ng * NTG:(ng + 1) * NTG, :], start=False, stop=True)
                    nc.scalar.activation(h1T[:, fc, :], h1[:, 0:NTG * 128],
                                         mybir.ActivationFunctionType.Relu,
                                         bias=bf1p[:, fc:fc + 1])
                for tt in range(NTG):
                    t = ng * NTG + tt
                    ps = ps_n.tile([128, 512], F32, tag="psn")
                    for fc in range(DFF // 128):
                        nc.tensor.matmul(ps[:, 0:D], h1T[:, fc, tt * 128:(tt + 1) * 128],
                                         Wf2[:, fc, :], start=(fc == 0), stop=False)
                    nc.tensor.matmul(ps[:, 0:D], ones[:], bf2[:], start=False, stop=True)
                    u = small.tile([128, D], F32, tag="u")
                    nc.vector.tensor_add(u[:], ps[:, 0:D], xln1_sb[:, t, :])
                    if l < L - 1:
                        layernorm_to(x_sb[:, t, :], u, g2, b2)
                    else:
                        layernorm_to(x_sb[:, t, :], u, g2, b2,
                                     extra_f32=n2p(h_out.ap())[:, t, :])

        for t in range(NT):
            pr = small.tile([128, D], F32, tag="hpr")
            nc.vector.tensor_mul(pr[:], x_sb[:, t, :], Wlk[:])
            nc.vector.reduce_sum(link_sb[:, t:t + 1], pr[:], axis=mybir.AxisListType.X)
            nc.vector.tensor_mul(pr[:], x_sb[:, t, :], Wcr[:])
            nc.vector.reduce_sum(crit_sb[:, t:t + 1], pr[:], axis=mybir.AxisListType.X)
        dma(link_out.ap().rearrange("(t p) -> p t", p=128), link_sb[:])
        dma(crit_out.ap().rearrange("(t p) -> p t", p=128), crit_sb[:])

    from concourse.library_overlay import lower_extended_insts
    lower_extended_insts(nc)
    _split_sync_waits(nc, mybir)
    return nc


# ==========================================================================
# numpy fallback (used only if the input graph violates our assumptions)
# ==========================================================================
def _np_reference(inp):
    SCALE = 1.0 / np.sqrt(HD)

    def ln(x, g, b, eps=1e-5):
        m = x.mean(-1, keepdims=True)
        v = ((x - m) ** 2).mean(-1, keepdims=True)
        return (x - m) / np.sqrt(v + eps) * g + b

    src, dst = inp['edge_index'][0], inp['edge_index'][1]
    res = np.concatenate([inp['power_alloc'], inp['beam_alloc']], axis=2).reshape(N, -1)
    inp0 = res @ np.asarray(inp['W_in']).T + inp['b_in']
    x = np.asarray(inp['node_power_attn']).reshape(N, -1) @ np.asarray(inp['W_emb']).T + inp['b_emb']
    epa = np.asarray(inp['edge_power_attn']).reshape(E, -1)
    for l in range(L):
        x = x + inp0
        ee = epa @ np.asarray(inp['We'])[l].T
        q = (x @ np.asarray(inp['Wq'])[l].T + inp['bq'][l])[dst].reshape(E, H, -1)
        k = ((x @ np.asarray(inp['Wk'])[l].T + inp['bk'][l])[src] + ee).reshape(E, H, -1)
        v = ((x @ np.asarray(inp['Wv'])[l].T + inp['bv'][l])[src] + ee).reshape(E, H, -1)
        alpha = (q * k).sum(-1) * SCALE
        m = np.full((N, H), -np.inf, np.float32)
        np.maximum.at(m, dst, alpha)
        ex = np.exp(alpha - m[dst])
        den = np.zeros((N, H), np.float32)
        np.add.at(den, dst, ex)
        a = ex / (den[dst] + 1e-16)
        agg = np.zeros((N, H, HC // H), np.float32)
        np.add.at(agg, dst, v * a[..., None])
        x2 = agg.reshape(N, -1) + x @ np.asarray(inp['Wskip'])[l].T + inp['bskip'][l]
        x = ln(x + x2, inp['ln1_g'][l], inp['ln1_b'][l])
        x2 = np.maximum(x @ np.asarray(inp['Wf1'])[l].T + inp['bf1'][l], 0) @ np.asarray(inp['Wf2'])[l].T + inp['bf2'][l]
        x = ln(x + x2, inp['ln2_g'][l], inp['ln2_b'][l])
    h_padded = x.reshape(B, MAXN, D).astype(np.float32)
    link_logits = (h_padded @ np.asarray(inp['W_link']).T)[..., 0] + np.asarray(inp['b_link'])[0]
    gm = x.reshape(B, MAXN, D).mean(axis=1)
    values = (gm @ np.asarray(inp['W_critic']).T)[:, 0] + np.asarray(inp['b_critic'])[0]
    mask = np.ones((B, MAXN), bool)
    return h_padded.astype(np.float32), link_logits.astype(np.float32), values.astype(np.float32), mask


# ==========================================================================
# entry point
# ==========================================================================
def kernel(**inputs):
    batch = np.asarray(inputs['batch'])
    ok = (batch.shape == (N,)) and np.array_equal(batch, np.arange(N) // MAXN)
    if not ok:
        print("kernel: unexpected batch structure; using host fallback", file=sys.stderr)
        return _np_reference(inputs)

    _apply_tile_patch()
    _apply_ntff_shim()
    from concourse.bass_utils import run_bass_kernel_spmd

    cfg, w, percore = _prep(inputs)
    key = (cfg['ET'],)
    if key not in _CACHE:
        _CACHE[key] = _build(cfg)
    nc = _CACHE[key]
    in_maps = [{**w, **pc} for pc in percore]
    trace = bool(int(os.environ.get('GNN_KERNEL_TRACE', '0')))
    tmpdir = os.environ.get('GNN_KERNEL_TRACE_DIR') or None
    res = run_bass_kernel_spmd(nc, in_maps, core_ids=list(range(NCORES)),
                               trace=trace, tmpdir=tmpdir)
    if trace:
        print(f"HW exec time: {res.exec_time_ns} ns")

    x = np.concatenate([res.results[c]['h_out'] for c in range(NCORES)], axis=0)
    link = np.concatenate([res.results[c]['link_out'] for c in range(NCORES)])
    crit = np.concatenate([res.results[c]['crit_out'] for c in range(NCORES)])
    h_padded = np.ascontiguousarray(x.reshape(B, MAXN, D), dtype=np.float32)
    link_logits = (link + np.float32(np.asarray(inputs['b_link'], np.float32)[0])).reshape(B, MAXN).astype(np.float32)
    values = (crit.reshape(B, MAXN).mean(axis=1) + np.asarray(inputs['b_critic'], np.float32)[0]).astype(np.float32)
    mask = np.ones((B, MAXN), bool)
    return h_padded, link_logits, values, mask


# revision 5
# speedup vs baseline: 1.1681x; 1.1681x over previous
"""Distributed Trainium2 (Bass) kernel for the gnn_message_passing problem.

Strategy (data-parallel over the B=64 equal graphs):
  - core c owns nodes [c*2048, (c+1)*2048) (8 graphs); edges are partitioned
    by the core that owns their *destination* and sorted by dst, so the
    segment softmax / scatter-add becomes a block-local one-hot matmul
    (fp8 one-hot adjacency blocks on the TensorEngine, fp32 PSUM accum).
  - per layer, the xk/xv node tables (bf16) are AllGather'd across the 8
    cores; per-edge k/v rows are fetched from the gathered table with SWDGE
    dma_gather (1KB rows); q rows are gathered from a core-local table.
  - edge features (epa @ We^T) are computed on the fly per 128-edge chunk
    with the TensorEngine; softmax runs without max-subtraction (activations
    are O(1) here; exp is evaluated in fp32 on ScalarE).

Host-side work (sharding, sorting, padding, bf16 casts, output assembly) is
numpy; all FLOP-heavy work runs on the 8 NeuronCores.
"""
import dataclasses
import os
import sys
import types
import contextlib
import ctypes
from contextlib import ExitStack

import numpy as np
import ml_dtypes

# hardcoded problem geometry (spec: nn_AC_88399016886986)
N, E, B, MAXN = 16384, 262144, 64, 256
D, HC, DFF, L = 256, 256, 1024, 3
H, HD = 8, 32
NCORES = 8

_CACHE = {}


# ==========================================================================
# harness patches (this container's walrus accepts only one inline sync wait
# per instruction; NTFF profiling hook is missing from the image's antenv)
# ==========================================================================
def _apply_tile_patch():
    import concourse.tile as tile
    from concourse import mybir
    from concourse.vector_clock import ScopedClock

    def _drain_and_barrier(self, tick_clock, wait_clock):
        probe = self.nc.sync.nop(nofuse=True)
        wait_clock.add_sem_waits(probe.ins, ScopedClock({None: tick_clock.global_clock}))
        si = probe.ins.sync_info
        waits = list(si.on_wait) if si and si.on_wait else []
        probe.ins.sync_info = mybir.SyncInfo(on_wait=waits[:1], on_update=[])
        for w in waits[1:]:
            n = self.nc.sync.nop(nofuse=True)
            n.ins.sync_info = mybir.SyncInfo(on_wait=[w], on_update=[])
        self.nc.sync.drain()
        self.nc.all_engine_barrier()
        assert self.sems is not None
        popped = self.nc._tile_sem_poison_stack.pop()
        assert popped is self._sem_poison
        self.nc.clear_and_free_semaphores(list(self.sems.allocated().values()))
        self.nc.all_engine_barrier()

    tile.TileContext._drain_and_barrier = _drain_and_barrier


def _apply_ntff_shim():
    if "antenv.axon_hooks" in sys.modules:
        return
    _SO = "/opt/axon/libaxon_pjrt.so"
    holder = [None]

    def _make_hook():
        try:
            lib = ctypes.CDLL(_SO)
        except OSError:
            return None
        if not hasattr(lib, "axon_start_nrt_profile"):
            return None
        lib.axon_start_nrt_profile.argtypes = [ctypes.POINTER(ctypes.c_int64), ctypes.c_size_t]
        lib.axon_start_nrt_profile.restype = ctypes.c_int64
        lib.axon_stop_nrt_profile.argtypes = [ctypes.c_char_p]
        lib.axon_stop_nrt_profile.restype = ctypes.c_int64

        @contextlib.contextmanager
        def _hook(output_dir, device_ids):
            import jax
            jax.devices()
            if device_ids:
                ids = (ctypes.c_int64 * len(device_ids))(*device_ids)
                rc = lib.axon_start_nrt_profile(ids, len(device_ids))
            else:
                rc = lib.axon_start_nrt_profile(None, 0)
            if rc != 0:
                raise RuntimeError(f"axon_start_nrt_profile rc={rc}")
            try:
                yield
            finally:
                nf = lib.axon_stop_nrt_profile(str(output_dir).encode())
                print(f"ntff profile: {nf} file(s) written to {output_dir}", file=sys.stderr)
        return _hook

    mod = types.ModuleType("antenv.axon_hooks")
    mod.get_axon_ntff_profile_hook = lambda: holder[0]
    mod.set_axon_ntff_profile_hook = lambda h: holder.__setitem__(0, h)
    sys.modules["antenv.axon_hooks"] = mod
    holder[0] = _make_hook()
    import concourse.bass_utils as bu
    bu.upload_artifacts = lambda tmpdir: "local://" + str(tmpdir)


def _split_sync_waits(nc, mybir, maxw=1):
    for f in nc.m.functions:
        for bb in f.blocks:
            newlist = []
            for inst in bb.instructions:
                si = inst.sync_info
                waits = list(si.on_wait) if si and si.on_wait else []
                if len(waits) > maxw:
                    extra, keep = waits[:-maxw], waits[-maxw:]
                    for i, w in enumerate(extra):
                        nop = mybir.InstNoOp(name=f"{inst.name}-ws{i}", ins=[], outs=[])
                        nop.engine = inst.engine
                        nop.sync_info = mybir.SyncInfo(on_wait=[w], on_update=[])
                        newlist.append(nop)
                        nc.register_instruction(nop, overwrite=True)
                    inst.sync_info = mybir.SyncInfo(
                        on_wait=keep,
                        on_update=list(si.on_update) if si.on_update else [])
                newlist.append(inst)
            bb.instructions[:] = newlist


# ==========================================================================
# host prep
# ==========================================================================
def _bf(x):
    return np.ascontiguousarray(np.asarray(x, np.float32)).astype(ml_dtypes.bfloat16)


def _wrap_idx(idx_flat):
    M = idx_flat.shape[0]
    w = idx_flat.reshape(M // 16, 16).T.astype(np.int16)
    return np.tile(w, (8, 1))


def _prep(inp):
    SCALE = 1.0 / np.sqrt(HD)
    NL = N // NCORES
    NT = NL // 128
    src = np.asarray(inp['edge_index'][0])
    dst = np.asarray(inp['edge_index'][1])
    epa = np.asarray(inp['edge_power_attn'], np.float32).reshape(E, -1)
    EDIM = epa.shape[1]

    res = np.concatenate([np.asarray(inp['power_alloc'], np.float32),
                          np.asarray(inp['beam_alloc'], np.float32)], axis=2).reshape(N, -1)
    npa = np.asarray(inp['node_power_attn'], np.float32).reshape(N, -1)

    cores = []
    ET = 0
    for c in range(NCORES):
        lo, hi = c * NL, (c + 1) * NL
        eids = np.nonzero((dst >= lo) & (dst < hi))[0]
        dl = dst[eids] - lo
        order = np.argsort(dl, kind='stable')
        eids, dl = eids[order], dl[order]
        counts = np.bincount(dl // 128, minlength=NT)
        ET = max(ET, int(counts.max()))
        cores.append((eids, dl, counts))
    ET = int(np.ceil(ET / 512) * 512)
    CT = ET // 128

    percore = []
    for c in range(NCORES):
        eids, dl, counts = cores[c]
        src_g = np.zeros((NT, ET), np.int64)
        qd_l = np.zeros((NT, ET), np.int64)
        A = np.zeros((NT, CT, 128, 128), np.float32)
        epaT = np.zeros((NT, EDIM, ET), np.float32)
        pos = 0
        for t in range(NT):
            cnt = counts[t]
            e_t = eids[pos:pos + cnt]
            d_t = dl[pos:pos + cnt]
            pos += cnt
            src_g[t, :cnt] = src[e_t]
            qd_l[t, :cnt] = d_t
            sl = np.arange(cnt)
            A[t, sl // 128, sl % 128, d_t - t * 128] = 1.0
            epaT[t, :, :cnt] = epa[e_t].T
        percore.append({
            'A': A.astype(ml_dtypes.float8_e4m3fn),
            'epaT': _bf(epaT),
            'srcidx': np.concatenate([_wrap_idx(src_g[t]) for t in range(NT)], axis=1),
            'qdstidx': np.concatenate([_wrap_idx(qd_l[t]) for t in range(NT)], axis=1),
            'resT': _bf(res[c * NL:(c + 1) * NL].T),
            'npaT': _bf(npa[c * NL:(c + 1) * NL].T),
        })

    w = {}
    w['WinTa'] = _bf(np.asarray(inp['W_in']).T[:128])
    w['WinTb'] = _bf(np.asarray(inp['W_in']).T[128:])
    w['bin'] = _bf(np.asarray(inp['b_in'])[None, :])
    w['WembTa'] = _bf(np.asarray(inp['W_emb']).T[:128])
    w['WembTb'] = _bf(np.asarray(inp['W_emb']).T[128:])
    w['bemb'] = _bf(np.asarray(inp['b_emb'])[None, :])
    w['WqT'] = _bf(np.asarray(inp['Wq']).transpose(0, 2, 1) * SCALE)
    w['bq'] = _bf(np.asarray(inp['bq'])[:, None, :] * SCALE)
    w['WkT'] = _bf(np.asarray(inp['Wk']).transpose(0, 2, 1))
    w['bk'] = _bf(np.asarray(inp['bk'])[:, None, :])
    w['WvT'] = _bf(np.asarray(inp['Wv']).transpose(0, 2, 1))
    w['bv'] = _bf(np.asarray(inp['bv'])[:, None, :])
    w['WskT'] = _bf(np.asarray(inp['Wskip']).transpose(0, 2, 1))
    w['bsk'] = _bf(np.asarray(inp['bskip'])[:, None, :])
    WeT = np.asarray(inp['We']).transpose(0, 2, 1)
    w['WeTa'] = _bf(WeT[:, :128])
    w['WeTb'] = _bf(WeT[:, 128:])
    w['Wf1T'] = _bf(np.asarray(inp['Wf1']).transpose(0, 2, 1))
    bf1 = np.asarray(inp['bf1'])
    w['bf1p'] = np.ascontiguousarray(
        bf1.reshape(L, DFF // 128, 128).transpose(0, 2, 1)).astype(np.float32)
    w['Wf2T'] = _bf(np.asarray(inp['Wf2']).transpose(0, 2, 1))
    w['bf2'] = _bf(np.asarray(inp['bf2'])[:, None, :])
    for nm in ('ln1_g', 'ln1_b', 'ln2_g', 'ln2_b'):
        w[nm] = _bf(np.tile(np.asarray(inp[nm])[:, None, :], (1, 128, 1)))
    w['Wlinkr'] = _bf(np.tile(np.asarray(inp['W_link'])[0][None, :], (128, 1)))
    w['Wcritr'] = _bf(np.tile(np.asarray(inp['W_critic'])[0][None, :], (128, 1)))
    w['ident'] = _bf(np.eye(128, dtype=np.float32))
    w['ones'] = _bf(np.ones((1, 128), np.float32))

    cfg = dict(NL=NL, NT=NT, ET=ET, CT=CT, EDIM=EDIM, INDIM=res.shape[1])
    return cfg, w, percore


# ==========================================================================
# bass graph
# ==========================================================================
def _build(cfg):
    import concourse.bass as bass
    import concourse.tile as tile
    from concourse import mybir
    from concourse import library_config

    BF16 = mybir.dt.bfloat16
    F32 = mybir.dt.float32
    FP8 = mybir.dt.float8e4
    I16 = mybir.dt.int16

    def bcast_last(ap, n):
        return dataclasses.replace(ap, ap=list(ap.ap) + [[0, n]])

    NL, NT, ET, CT = cfg['NL'], cfg['NT'], cfg['ET'], cfg['CT']
    EDIM = cfg['EDIM']
    EB = EDIM - 128
    GRP = 4
    assert CT % GRP == 0
    NG = CT // GRP
    IDXW = ET // 16
    NTG = 2

    nc = bass.Bass("TRN2", target_bir_lowering=False, debug=False, num_devices=NCORES)

    def par(name, shape, dt):
        return nc.declare_dram_parameter(name, list(shape), dt, isOutput=False)

    A_d = par('A', (NT, CT, 128, 128), FP8)
    epaT_d = par('epaT', (NT, EDIM, ET), BF16)
    srcidx_d = par('srcidx', (128, NT * IDXW), I16)
    qdstidx_d = par('qdstidx', (128, NT * IDXW), I16)
    resT_d = par('resT', (cfg['INDIM'], NL), BF16)
    npaT_d = par('npaT', (cfg['INDIM'], NL), BF16)
    wd = {}
    for nm, shape, dt in [
        ('WinTa', (128, D), BF16), ('WinTb', (cfg['INDIM'] - 128, D), BF16), ('bin', (1, D), BF16),
        ('WembTa', (128, D), BF16), ('WembTb', (cfg['INDIM'] - 128, D), BF16), ('bemb', (1, D), BF16),
        ('WqT', (L, D, HC), BF16), ('bq', (L, 1, HC), BF16),
        ('WkT', (L, D, HC), BF16), ('bk', (L, 1, HC), BF16),
        ('WvT', (L, D, HC), BF16), ('bv', (L, 1, HC), BF16),
        ('WskT', (L, D, HC), BF16), ('bsk', (L, 1, HC), BF16),
        ('WeTa', (L, 128, HC), BF16), ('WeTb', (L, EB, HC), BF16),
        ('Wf1T', (L, D, DFF), BF16), ('bf1p', (L, 128, DFF // 128), F32),
        ('Wf2T', (L, DFF, D), BF16), ('bf2', (L, 1, D), BF16),
        ('ln1_g', (L, 128, D), BF16), ('ln1_b', (L, 128, D), BF16),
        ('ln2_g', (L, 128, D), BF16), ('ln2_b', (L, 128, D), BF16),
        ('Wlinkr', (128, D), BF16), ('Wcritr', (128, D), BF16),
        ('ident', (128, 128), BF16), ('ones', (1, 128), BF16),
    ]:
        wd[nm] = par(nm, shape, dt)

    h_out = nc.declare_dram_parameter('h_out', [NL, D], F32, isOutput=True)
    link_out = nc.declare_dram_parameter('link_out', [NL], F32, isOutput=True)
    crit_out = nc.declare_dram_parameter('crit_out', [NL], F32, isOutput=True)

    xq_dram = nc.dram_tensor('xq_dram', [NL, HC], BF16)
    cc_in = nc.dram_tensor('cc_in', [NL, 2 * HC], BF16)
    xkv_full = nc.dram_tensor('xkv_full', [N, 2 * HC], BF16, addr_space="Shared")

    n2p = lambda ap: ap.rearrange("(t p) f -> p t f", p=128)

    with ExitStack() as ctx:
        tc = ctx.enter_context(tile.TileContext(nc))
        nc.gpsimd.load_library(library_config.mlp)

        const = ctx.enter_context(tc.tile_pool(name="const", bufs=1))
        wpool = ctx.enter_context(tc.tile_pool(name="wpool", bufs=1))
        state = ctx.enter_context(tc.tile_pool(name="state", bufs=1))
        nwork = ctx.enter_context(tc.tile_pool(name="nwork", bufs=2))
        small = ctx.enter_context(tc.tile_pool(name="small", bufs=1))
        ps_n = ctx.enter_context(tc.tile_pool(name="ps_n", bufs=3, space="PSUM"))
        ps_ee = ctx.enter_context(tc.tile_pool(name="ps_ee", bufs=2, space="PSUM"))
        ps_ag = ctx.enter_context(tc.tile_pool(name="ps_ag", bufs=1, space="PSUM"))

        dma = nc.sync.dma_start

        ident = const.tile([128, 128], BF16)
        dma(ident[:], wd['ident'][:])
        ones = const.tile([1, 128], BF16)
        dma(ones[:], wd['ones'][:])
        srcidx = const.tile([128, NT * IDXW], I16)
        dma(srcidx[:], srcidx_d[:])
        qdstidx = const.tile([128, NT * IDXW], I16)
        dma(qdstidx[:], qdstidx_d[:])
        Wlk = const.tile([128, D], BF16)
        dma(Wlk[:], wd['Wlinkr'][:])
        Wcr = const.tile([128, D], BF16)
        dma(Wcr[:], wd['Wcritr'][:])

        x_sb = state.tile([128, NT, D], BF16)
        inp0_sb = state.tile([128, NT, D], BF16)
        xin_sb = state.tile([128, NT, D], BF16)
        xinT_sb = state.tile([128, 2, NT, 128], BF16)
        xsk_sb = state.tile([128, NT, HC], BF16)
        xln1_sb = state.tile([128, NT, D], BF16)
        xln1T_sb = state.tile([128, 2, NT, 128], BF16)
        link_sb = state.tile([128, NT], F32)
        crit_sb = state.tile([128, NT], F32)

        def layernorm_to(dst_ap, u, gam, bet, extra_f32=None):
            mu = small.tile([128, 1], F32, tag="mu")
            nc.vector.reduce_sum(mu[:], u[:], axis=mybir.AxisListType.X)
            nc.vector.tensor_scalar_mul(mu[:], mu[:], 1.0 / D)
            cen = small.tile([128, D], F32, tag="cen")
            nc.vector.tensor_scalar(cen[:], u[:], mu[:], None, mybir.AluOpType.subtract)
            sq = small.tile([128, D], F32, tag="sq")
            nc.scalar.square(sq[:], cen[:])
            var = small.tile([128, 1], F32, tag="var")
            nc.vector.reduce_sum(var[:], sq[:], axis=mybir.AxisListType.X)
            nc.vector.tensor_scalar(var[:], var[:], 1.0 / D, 1e-5,
                                    mybir.AluOpType.mult, mybir.AluOpType.add)
            rstd = small.tile([128, 1], F32, tag="rstd")
            nc.scalar.sqrt(rstd[:], var[:])
            nc.vector.reciprocal(rstd[:], rstd[:])
            xl = small.tile([128, D], BF16, tag="xl")
            nc.vector.tensor_scalar_mul(xl[:], cen[:], rstd[:])
            nc.vector.tensor_mul(xl[:], xl[:], gam[:])
            nc.vector.tensor_add(dst_ap, xl[:], bet[:])
            if extra_f32 is not None:
                xf = small.tile([128, D], F32, tag="xf")
                nc.vector.tensor_add(xf[:], xl[:], bet[:])
                dma(extra_f32, xf[:])

        # phase 0: input embeddings (pool freed before the edge-phase pools)
        embpool = tc.tile_pool(name="emb", bufs=1)
        emb = embpool.__enter__()
        emb_in = emb.tile([128, 2, NL], BF16)
        embB_in = emb.tile([EB, 2, NL], BF16)
        dma(emb_in[:, 0, :], resT_d[0:128, :])
        dma(embB_in[:, 0, :], resT_d[128:EDIM, :])
        dma(emb_in[:, 1, :], npaT_d[0:128, :])
        dma(embB_in[:, 1, :], npaT_d[128:EDIM, :])
        embW = emb.tile([128, 2, D], BF16)
        embWB = emb.tile([EB, 2, D], BF16)
        dma(embW[:, 0, :], wd['WinTa'][:])
        dma(embWB[:, 0, :], wd['WinTb'][:])
        dma(embW[:, 1, :], wd['WembTa'][:])
        dma(embWB[:, 1, :], wd['WembTb'][:])
        embb = emb.tile([1, 2, D], BF16)
        dma(embb[:, 0, :], wd['bin'][:])
        dma(embb[:, 1, :], wd['bemb'][:])
        for t in range(NT):
            for j, dstt in ((0, inp0_sb), (1, x_sb)):
                ps = ps_n.tile([128, 512], F32, tag="psn")
                nc.tensor.matmul(ps[:, 0:D], emb_in[:, j, t * 128:(t + 1) * 128], embW[:, j, :], start=True, stop=False)
                nc.tensor.matmul(ps[:, 0:D], embB_in[:, j, t * 128:(t + 1) * 128], embWB[:, j, :], start=False, stop=False)
                nc.tensor.matmul(ps[:, 0:D], ones[:], embb[:, j, :], start=False, stop=True)
                nc.scalar.copy(dstt[:, t, :], ps[:, 0:D])
        embpool.__exit__(None, None, None)
        ework = ctx.enter_context(tc.tile_pool(name="ework", bufs=2))
        gwork = ctx.enter_context(tc.tile_pool(name="gwork", bufs=2))

        for l in range(L):
            WqT = wpool.tile([128, 2, HC], BF16, tag="wq")
            WkT = wpool.tile([128, 2, HC], BF16, tag="wk")
            WvT = wpool.tile([128, 2, HC], BF16, tag="wv")
            WskT = wpool.tile([128, 2, HC], BF16, tag="wsk")
            for tl, nm in ((WqT, 'WqT'), (WkT, 'WkT'), (WvT, 'WvT'), (WskT, 'WskT')):
                dma(tl[:], wd[nm][l].rearrange("(c p) f -> p c f", p=128))
            biases = wpool.tile([1, 4, HC], BF16, tag="wb")
            for j, nm in enumerate(('bq', 'bk', 'bv', 'bsk')):
                dma(biases[:, j, :], wd[nm][l])
            WeA = wpool.tile([128, HC], BF16, tag="wea")
            dma(WeA[:], wd['WeTa'][l])
            WeB = wpool.tile([EB, HC], BF16, tag="web")
            dma(WeB[:], wd['WeTb'][l])
            Wf1 = wpool.tile([128, 2, DFF], BF16, tag="wf1")
            dma(Wf1[:], wd['Wf1T'][l].rearrange("(c p) f -> p c f", p=128))
            bf1p = wpool.tile([128, DFF // 128], F32, tag="wbf1")
            dma(bf1p[:], wd['bf1p'][l])
            Wf2 = wpool.tile([128, DFF // 128, D], BF16, tag="wf2")
            dma(Wf2[:], wd['Wf2T'][l].rearrange("(c p) f -> p c f", p=128))
            bf2 = wpool.tile([1, D], BF16, tag="wbf2")
            dma(bf2[:], wd['bf2'][l])
            g1 = wpool.tile([128, D], BF16, tag="g1")
            dma(g1[:], wd['ln1_g'][l])
            b1 = wpool.tile([128, D], BF16, tag="b1")
            dma(b1[:], wd['ln1_b'][l])
            g2 = wpool.tile([128, D], BF16, tag="g2")
            dma(g2[:], wd['ln2_g'][l])
            b2 = wpool.tile([128, D], BF16, tag="b2")
            dma(b2[:], wd['ln2_b'][l])

            # node phase
            nc.vector.tensor_add(xin_sb[:], x_sb[:], inp0_sb[:])
            for t in range(NT):
                for dc in range(2):
                    pst = ps_n.tile([128, 512], BF16, tag="psn")
                    nc.tensor.transpose(pst[:, 0:128], xin_sb[:, t, dc * 128:(dc + 1) * 128], ident[:])
                    nc.scalar.copy(xinT_sb[:, dc, t, :], pst[:, 0:128])
            for t in range(NT):
                kvt = nwork.tile([128, 2 * HC], BF16, tag="kvt")
                xqt = nwork.tile([128, HC], BF16, tag="xqt")
                for j, W in ((0, WqT), (3, WskT), (1, WkT), (2, WvT)):
                    ps = ps_n.tile([128, 512], F32, tag="psn")
                    nc.tensor.matmul(ps[:, 0:HC], xinT_sb[:, 0, t, :], W[:, 0, :], start=True, stop=False)
                    nc.tensor.matmul(ps[:, 0:HC], xinT_sb[:, 1, t, :], W[:, 1, :], start=False, stop=False)
                    nc.tensor.matmul(ps[:, 0:HC], ones[:], biases[:, j, :], start=False, stop=True)
                    if j == 0:
                        nc.scalar.copy(xqt[:], ps[:, 0:HC])
                    elif j == 3:
                        nc.scalar.copy(xsk_sb[:, t, :], ps[:, 0:HC])
                    elif j == 1:
                        nc.scalar.copy(kvt[:, 0:HC], ps[:, 0:HC])
                    else:
                        nc.scalar.copy(kvt[:, HC:2 * HC], ps[:, 0:HC])
                dma(n2p(cc_in.ap())[:, t, :], kvt[:])
                dma(n2p(xq_dram.ap())[:, t, :], xqt[:])
            nc.gpsimd.collective_compute(
                "AllGather", mybir.AluOpType.bypass,
                replica_groups=[list(range(NCORES))],
                ins=[cc_in.ap().opt()], outs=[xkv_full.ap().opt()])

            # edge phase
            et_reg = nc.gpsimd.snap(ET)
            for t in range(NT):
                kvg = gwork.tile([128, CT, 2 * HC], BF16, tag="kvg")
                nc.gpsimd.dma_gather(
                    out_ap=kvg[:], in_ap=xkv_full.ap(),
                    idxs_ap=srcidx[:, t * IDXW:(t + 1) * IDXW],
                    num_idxs=ET, num_idxs_reg=et_reg, elem_size=2 * HC, single_packet=False)
                qg = gwork.tile([128, CT, HC], BF16, tag="qg")
                nc.gpsimd.dma_gather(
                    out_ap=qg[:], in_ap=xq_dram.ap(),
                    idxs_ap=qdstidx[:, t * IDXW:(t + 1) * IDXW],
                    num_idxs=ET, num_idxs_reg=et_reg, elem_size=HC, single_packet=False)
                epaA = gwork.tile([128, ET], BF16, tag="epaA")
                dma(epaA[:], epaT_d[t, 0:128, :])
                epaB = gwork.tile([EB, ET], BF16, tag="epaB")
                dma(epaB[:], epaT_d[t, 128:EDIM, :])
                A_t = gwork.tile([128, CT, 128], FP8, tag="At")
                dma(A_t[:], A_d[t].rearrange("c p n -> p c n"))

                agg = ps_ag.tile([128, HC + H], F32, tag="agg")
                for g in range(NG):
                    ee = ps_ee.tile([128, GRP, HC], F32, tag="ee")
                    for j in range(GRP):
                        cch = g * GRP + j
                        sl = slice(cch * 128, (cch + 1) * 128)
                        nc.tensor.matmul(ee[:, j, :], epaA[:, sl], WeA[:], start=True, stop=False)
                        nc.tensor.matmul(ee[:, j, :], epaB[:, sl], WeB[:], start=False, stop=True)
                    gs = slice(g * GRP, (g + 1) * GRP)
                    ee_sb = ework.tile([128, GRP, HC], BF16, tag="eesb")
                    nc.scalar.copy(ee_sb[:], ee[:])
                    k_sb = ework.tile([128, GRP, HC], BF16, tag="k")
                    nc.vector.tensor_add(k_sb[:], ee_sb[:], kvg[:, gs, 0:HC])
                    v_sb = ework.tile([128, GRP, HC], BF16, tag="v")
                    nc.vector.tensor_add(v_sb[:], ee_sb[:], kvg[:, gs, HC:2 * HC])
                    prod = k_sb
                    nc.vector.tensor_mul(prod[:], k_sb[:], qg[:, gs, :])
                    alph = ework.tile([128, GRP, H], F32, tag="alph")
                    nc.vector.reduce_sum(
                        alph[:],
                        prod[:].rearrange("p g (h d) -> p g h d", h=H),
                        axis=mybir.AxisListType.X)
                    wex = ework.tile([128, GRP, HC + H], BF16, tag="wex")
                    nc.scalar.activation(wex[:, :, HC:HC + H], alph[:],
                                         mybir.ActivationFunctionType.Exp)
                    nc.vector.tensor_mul(
                        wex[:, :, 0:HC].rearrange("p g (h d) -> p g h d", h=H),
                        v_sb[:].rearrange("p g (h d) -> p g h d", h=H),
                        bcast_last(wex[:, :, HC:HC + H], HD))
                    for j in range(GRP):
                        cch = g * GRP + j
                        nc.tensor.matmul(agg[:], A_t[:, cch, :], wex[:, j, :],
                                         start=(cch == 0), stop=(cch == CT - 1))

                dinv = small.tile([128, H], F32, tag="dinv")
                nc.vector.tensor_scalar_add(dinv[:], agg[:, HC:HC + H], 1e-16)
                nc.vector.reciprocal(dinv[:], dinv[:])
                u = small.tile([128, D], F32, tag="u")
                nc.vector.tensor_mul(
                    u[:].rearrange("p (h d) -> p h d", h=H),
                    agg[:, 0:HC].rearrange("p (h d) -> p h d", h=H),
                    bcast_last(dinv[:], HD))
                nc.vector.tensor_add(u[:], u[:], xsk_sb[:, t, :])
                nc.vector.tensor_add(u[:], u[:], xin_sb[:, t, :])
                layernorm_to(xln1_sb[:, t, :], u, g1, b1)
                for dc in range(2):
                    pst = ps_n.tile([128, 512], BF16, tag="psn")
                    nc.tensor.transpose(pst[:, 0:128], xln1_sb[:, t, dc * 128:(dc + 1) * 128], ident[:])
                    nc.scalar.copy(xln1T_sb[:, dc, t, :], pst[:, 0:128])

            # FFN
            for ng in range(NT // NTG):
                h1T = nwork.tile([128, DFF // 128, NTG * 128], BF16, tag="h1T")
                for fc in range(DFF // 128):
                    h1 = ps_n.tile([128, 512], F32, tag="psn")
                    nc.tensor.matmul(h1[:, 0:NTG * 128], Wf1[:, 0, fc * 128:(fc + 1) * 128],
                                     xln1T_sb[:, 0, ng * NTG:(ng + 1) * NTG, :], start=True, stop=False)
                    nc.tensor.matmul(h1[:, 0:NTG * 128], Wf1[:, 1, fc * 128:(fc + 1) * 128],
                                     xln1T_sb[:, 1, ng * NTG:(ng + 1) * NTG, :], start=False, stop=True)
                    nc.scalar.activation(h1T[:, fc, :], h1[:, 0:NTG * 128],
                                         mybir.ActivationFunctionType.Relu,
                                         bias=bf1p[:, fc:fc + 1])
                for tt in range(NTG):
                    t = ng * NTG + tt
                    ps = ps_n.tile([128, 512], F32, tag="psn")
                    for fc in range(DFF // 128):
                        nc.tensor.matmul(ps[:, 0:D], h1T[:, fc, tt * 128:(tt + 1) * 128],
                                         Wf2[:, fc, :], start=(fc == 0), stop=False)
                    nc.tensor.matmul(ps[:, 0:D], ones[:], bf2[:], start=False, stop=True)
                    u = small.tile([128, D], F32, tag="u")
                    nc.vector.tensor_add(u[:], ps[:, 0:D], xln1_sb[:, t, :])
                    if l < L - 1:
                        layernorm_to(x_sb[:, t, :], u, g2, b2)
                    else:
                        layernorm_to(x_sb[:, t, :], u, g2, b2,
                                     extra_f32=n2p(h_out.ap())[:, t, :])

        for t in range(NT):
            pr = small.tile([128, D], F32, tag="hpr")
            nc.vector.tensor_mul(pr[:], x_sb[:, t, :], Wlk[:])
            nc.vector.reduce_sum(link_sb[:, t:t + 1], pr[:], axis=mybir.AxisListType.X)
            nc.vector.tensor_mul(pr[:], x_sb[:, t, :], Wcr[:])
            nc.vector.reduce_sum(crit_sb[:, t:t + 1], pr[:], axis=mybir.AxisListType.X)
        dma(link_out.ap().rearrange("(t p) -> p t", p=128), link_sb[:])
        dma(crit_out.ap().rearrange("(t p) -> p t", p=128), crit_sb[:])

    from concourse.library_overlay import lower_extended_insts
    lower_extended_insts(nc)
    _split_sync_waits(nc, mybir)
    return nc


# ==========================================================================
# numpy fallback (used only if the input graph violates our assumptions)
# ==========================================================================
def _np_reference(inp):
    SCALE = 1.0 / np.sqrt(HD)

    def ln(x, g, b, eps=1e-5):
        m = x.mean(-1, keepdims=True)
        v = ((x - m) ** 2).mean(-1, keepdims=True)
        return (x - m) / np.sqrt(v + eps) * g + b

    src, dst = inp['edge_index'][0], inp['edge_index'][1]
    res = np.concatenate([inp['power_alloc'], inp['beam_alloc']], axis=2).reshape(N, -1)
    inp0 = res @ np.asarray(inp['W_in']).T + inp['b_in']
    x = np.asarray(inp['node_power_attn']).reshape(N, -1) @ np.asarray(inp['W_emb']).T + inp['b_emb']
    epa = np.asarray(inp['edge_power_attn']).reshape(E, -1)
    for l in range(L):
        x = x + inp0
        ee = epa @ np.asarray(inp['We'])[l].T
        q = (x @ np.asarray(inp['Wq'])[l].T + inp['bq'][l])[dst].reshape(E, H, -1)
        k = ((x @ np.asarray(inp['Wk'])[l].T + inp['bk'][l])[src] + ee).reshape(E, H, -1)
        v = ((x @ np.asarray(inp['Wv'])[l].T + inp['bv'][l])[src] + ee).reshape(E, H, -1)
        alpha = (q * k).sum(-1) * SCALE
        m = np.full((N, H), -np.inf, np.float32)
        np.maximum.at(m, dst, alpha)
        ex = np.exp(alpha - m[dst])
        den = np.zeros((N, H), np.float32)
        np.add.at(den, dst, ex)
        a = ex / (den[dst] + 1e-16)
        agg = np.zeros((N, H, HC // H), np.float32)
        np.add.at(agg, dst, v * a[..., None])
        x2 = agg.reshape(N, -1) + x @ np.asarray(inp['Wskip'])[l].T + inp['bskip'][l]
        x = ln(x + x2, inp['ln1_g'][l], inp['ln1_b'][l])
        x2 = np.maximum(x @ np.asarray(inp['Wf1'])[l].T + inp['bf1'][l], 0) @ np.asarray(inp['Wf2'])[l].T + inp['bf2'][l]
        x = ln(x + x2, inp['ln2_g'][l], inp['ln2_b'][l])
    h_padded = x.reshape(B, MAXN, D).astype(np.float32)
    link_logits = (h_padded @ np.asarray(inp['W_link']).T)[..., 0] + np.asarray(inp['b_link'])[0]
    gm = x.reshape(B, MAXN, D).mean(axis=1)
    values = (gm @ np.asarray(inp['W_critic']).T)[:, 0] + np.asarray(inp['b_critic'])[0]
    mask = np.ones((B, MAXN), bool)
    return h_padded.astype(np.float32), link_logits.astype(np.float32), values.astype(np.float32), mask


# ==========================================================================
# entry point
# ==========================================================================
def kernel(**inputs):
    batch = np.asarray(inputs['batch'])
    ok = (batch.shape == (N,)) and np.array_equal(batch, np.arange(N) // MAXN)
    if not ok:
        print("kernel: unexpected batch structure; using host fallback", file=sys.stderr)
        return _np_reference(inputs)

    _apply_tile_patch()
    _apply_ntff_shim()
    from concourse.bass_utils import run_bass_kernel_spmd

    cfg, w, percore = _prep(inputs)
    key = (cfg['ET'],)
    if key not in _CACHE:
        _CACHE[key] = _build(cfg)
    nc = _CACHE[key]
    in_maps = [{**w, **pc} for pc in percore]
    trace = bool(int(os.environ.get('GNN_KERNEL_TRACE', '0')))
    tmpdir = os.environ.get('GNN_KERNEL_TRACE_DIR') or None
    res = run_bass_kernel_spmd(nc, in_maps, core_ids=list(range(NCORES)),
                               trace=trace, tmpdir=tmpdir)
    if trace:
        print(f"HW exec time: {res.exec_time_ns} ns")

    x = np.concatenate([res.results[c]['h_out'] for c in range(NCORES)], axis=0)
    link = np.concatenate([res.results[c]['link_out'] for c in range(NCORES)])
    crit = np.concatenate([res.results[c]['crit_out'] for c in range(NCORES)])
    h_padded = np.ascontiguousarray(x.reshape(B, MAXN, D), dtype=np.float32)
    link_logits = (link + np.float32(np.asarray(inputs['b_link'], np.float32)[0])).reshape(B, MAXN).astype(np.float32)
    values = (crit.reshape(B, MAXN).mean(axis=1) + np.asarray(inputs['b_critic'], np.float32)[0]).astype(np.float32)
    mask = np.ones((B, MAXN), bool)
    return h_padded, link_logits, values, mask
